# revision 22
# baseline (speedup 1.0000x reference)
"""GCN (2-layer, PyG GCNConv-style) on 8 Trainium2 NeuronCores.

Strategy (sharding_hint): nodes sharded across the 8 cores (data parallel on
the node dim); edges partitioned by destination core so the scatter-add stays
local; per layer the dinv-scaled transformed features are AllGathered so each
core can gather arbitrary source rows; weights replicated.

Math (per layer, A' = A + I, dinv = deg^-1/2):
    out = dinv . (A'^T (dinv . (x @ W))) + b
We fold norms so no per-edge scaling is needed:
  - table  = dinv . (x @ W)                    (per-node scale, ACT)
  - agg    = A^T table + table + b * sqrtdeg   (PE one-hot matmuls; the
             self-loop term "+ table" is a dense per-tile transpose matmul,
             so self loops never enter the gathered edge streams)
  - layer1 h2 = dinv . relu(agg)               (dinv moved past relu, dinv>0)
    the dinv is then folded into layer2's table scale (dinv^2).
  - layer2 out = dinv . agg2                   (final per-node scale)

Edge aggregation: edges are grouped on host by (dst-tile t of 128 nodes,
src-group g of 32768 nodes); per (g,t) the segment is padded to a multiple of
16 only (pad idx=0, pad dst label -1).  Segments are concatenated per
(g, tile-block) into one gather call; matmul chunks of 128 gathered rows may
straddle segment boundaries, which is handled with partition-sliced one-hot
matmuls (the straddle map is static because segment sizes are shared across
cores; per-core count variation hides in the dst labels, where -1 kills the
one-hot).  Scatter-add accumulates directly into one PSUM tile per dst tile.
"""

import functools
import numpy as np

import concourse.bacc as bacc
import concourse.mybir as mybir
import concourse.tile as tile
from concourse.bass_utils import run_bass_kernel_spmd
from concourse.masks import make_identity

NCORE = 8
P = 128
GSHIFT = 15  # src-group size 32768 (int16 index range)
GSZ = 1 << GSHIFT
TBSZ = 4  # dst tiles per gather block

F16 = mybir.dt.float16
F32 = mybir.dt.float32
I16 = mybir.dt.int16


def _round_up(a, b):
    return (a + b - 1) // b * b


def _call_layout(ecnt, blocks, NG):
    """Static stream layout.  Returns per-call info:
    calls[(bi, g)] = dict(B, off16, offch, cov) where cov[k] = list of
    (t, lo, hi) partition-slices of chunk k."""
    calls = {}
    off16 = 0  # global gidx offset, units of 16 idxs
    offch = 0  # global chunk offset (dloc columns)
    for bi, blk in enumerate(blocks):
        for g in range(NG):
            segs = []
            s = 0
            for t in blk:
                e = ecnt[g][t]
                segs.append((t, s, e))
                s += e
            B = s
            K = -(-B // P) if B else 0
            cov = []
            for k in range(K):
                lo_k, hi_k = k * P, (k + 1) * P
                entries = []
                for t, st, e in segs:
                    if e == 0:
                        continue
                    a, b = max(st, lo_k), min(st + e, hi_k)
                    if a < b:
                        entries.append([t, a - lo_k, b - lo_k])
                cov.append(entries)
            calls[(bi, g)] = dict(B=B, off16=off16, offch=offch, cov=cov)
            off16 += B // 16
            offch += K
    return calls, off16 * 16, offch


# ----------------------------------------------------------------------------
# Bass program (identical for all 8 cores; per-core data differs via inputs)
# ----------------------------------------------------------------------------


def _build(cfg):
    """cfg = (F, H, C, NS, ecnt) with ecnt[g][t] = padded16 edge count."""
    Fdim, H, C, NS, ecnt = cfg
    ecnt = [list(gr) for gr in ecnt]
    T = NS // P
    NPAD = NCORE * NS
    NG = len(ecnt)
    blocks = [list(range(b, min(b + TBSZ, T))) for b in range(0, T, TBSZ)]
    calls, E_IDX, NCH = _call_layout(ecnt, blocks, NG)
    SCMAX = max(len(c["cov"]) for c in calls.values())

    nc = bacc.Bacc(None, target_bir_lowering=False)

    # ---- I/O ----
    xT_in = nc.dram_tensor("xT", [P, NS], F32, kind="ExternalInput")
    degnm_in = nc.dram_tensor("deg_nm", [P, T], F32, kind="ExternalInput")
    degrow_in = nc.dram_tensor("deg_row", [1, NS], F32, kind="ExternalInput")
    w1_in = nc.dram_tensor("W1", [Fdim, H], F32, kind="ExternalInput")
    w2_in = nc.dram_tensor("W2", [H, C], F32, kind="ExternalInput")
    b1_in = nc.dram_tensor("b1", [1, H], F32, kind="ExternalInput")
    b2_in = nc.dram_tensor("b2", [1, C], F32, kind="ExternalInput")
    gidx_in = nc.dram_tensor("gidx", [P, E_IDX // 16], I16, kind="ExternalInput")
    dloc_in = nc.dram_tensor("dloc", [P, NCH], F16, kind="ExternalInput")
    out_ext = nc.dram_tensor("out_nm", [NS, C], F32, kind="ExternalOutput")

    hsh = nc.dram_tensor("hsh", [NS, H], F16)
    gsh = nc.dram_tensor("gsh", [NS, C], F16)
    hfull = nc.dram_tensor("hfull", [NPAD, P], F16)
    gfull = nc.dram_tensor("gfull", [NPAD, P], F16)
    hfull64 = nc.dram_tensor("hfull64", [NPAD, H], F16, addr_space="Shared")
    gfull40 = nc.dram_tensor("gfull40", [NPAD, C], F16, addr_space="Shared")
    rgroups = [list(range(NCORE))]

    with tile.TileContext(nc) as tc:
        with (
            tc.tile_pool(name="con", bufs=1) as con,
            tc.tile_pool(name="meta", bufs=1) as meta,
            tc.tile_pool(name="stg", bufs=1) as stg,
            tc.tile_pool(name="io", bufs=3) as io,
            tc.tile_pool(name="eb", bufs=3) as eb,
            tc.tile_pool(name="rs", bufs=2) as rs,
            tc.tile_pool(name="ps", bufs=1, space="PSUM") as ps,
            tc.tile_pool(name="pst", bufs=2, space="PSUM") as pst,
            tc.tile_pool(name="pst2", bufs=2, space="PSUM") as pst2,
        ):
            # ---- Phase A: constants / metadata ----
            dloc = meta.tile([P, NCH], F16)
            nc.sync.dma_start(dloc[:], dloc_in[:])

            w1f = con.tile([Fdim, H], F32)
            nc.sync.dma_start(w1f[:], w1_in[:])
            w1 = con.tile([Fdim, H], F16)
            nc.vector.tensor_copy(w1[:], w1f[:])
            w2f = con.tile([H, C], F32)
            nc.sync.dma_start(w2f[:], w2_in[:])
            w2 = con.tile([H, C], F16)
            nc.vector.tensor_copy(w2[:], w2f[:])
            b1f = con.tile([1, H], F32)
            nc.sync.dma_start(b1f[:], b1_in[:])
            b1 = con.tile([1, H], F16)
            nc.vector.tensor_copy(b1[:], b1f[:])
            b2f = con.tile([1, C], F32)
            nc.sync.dma_start(b2f[:], b2_in[:])
            b2 = con.tile([1, C], F16)
            nc.vector.tensor_copy(b2[:], b2f[:])

            degnm = con.tile([P, T], F32)
            nc.sync.dma_start(degnm[:], degnm_in[:])
            sq_nm = con.tile([P, T], F32)
            nc.scalar.activation(sq_nm[:], degnm[:], mybir.ActivationFunctionType.Sqrt)
            dinv_nm = con.tile([P, T], F32)
            nc.vector.reciprocal(dinv_nm[:], sq_nm[:])
            dinv2_nm = con.tile([P, T], F32)
            nc.vector.tensor_mul(dinv2_nm[:], dinv_nm[:], dinv_nm[:])

            degrow = con.tile([1, NS], F32)
            nc.sync.dma_start(degrow[:], degrow_in[:])
            sqrow = con.tile([1, NS], F16)
            nc.scalar.activation(sqrow[:], degrow[:], mybir.ActivationFunctionType.Sqrt)

            iota_i = con.tile([P, P], I16)
            nc.gpsimd.iota(iota_i[:], pattern=[[1, P]], base=0, channel_multiplier=0)
            iota16 = con.tile([P, P], F16)
            nc.vector.tensor_copy(iota16[:], iota_i[:])

            ident = con.tile([P, P], F32)
            make_identity(nc, ident[:])
            ident16 = con.tile([P, P], F16)
            nc.vector.tensor_copy(ident16[:], ident[:])

            stage = stg.tile([P, T, H], F16, tag="stage")
            stage2 = stg.tile([P, T, C], F16, tag="stage2")

            # ---- Phase B: layer-1 transform, build h' table ----
            for t in range(T):
                xt = io.tile([P, P], F32, tag="xt")
                nc.sync.dma_start(xt[:], xT_in[:, t * P : (t + 1) * P])
                xt16 = io.tile([P, P], F16, tag="xt16")
                nc.vector.tensor_copy(xt16[:], xt[:])
                ph = pst.tile([P, H], F32, tag="pt")
                nc.tensor.matmul(ph[:], xt16[:], w1[:], start=True, stop=True)
                nc.scalar.activation(
                    stage[:, t, 0:H],
                    ph[:],
                    mybir.ActivationFunctionType.Copy,
                    scale=dinv_nm[:, t : t + 1],
                )
            nc.sync.dma_start(hsh.rearrange("(t p) d -> p t d", p=P)[:], stage[:])

            # ---- Phase C: AllGather layer-1 table ----
            nc.gpsimd.collective_compute(
                "AllGather",
                mybir.AluOpType.bypass,
                ins=[hsh[:]],
                outs=[hfull64[:]],
                replica_groups=rgroups,
            )
            for gb in range(0, NPAD, 8192):
                ge = min(gb + 8192, NPAD)
                bt = rs.tile([P, 64, H], F16, tag="restride")
                nw = (ge - gb) // P
                nc.scalar.dma_start(
                    bt[:, 0:nw, :],
                    hfull64[gb:ge, :].rearrange("(c p) e -> p c e", p=P),
                )
                nc.scalar.dma_start(
                    hfull[gb:ge, 0:H].rearrange("(c p) e -> p c e", p=P),
                    bt[:, 0:nw, :],
                )

            rt16 = stg.tile([H, T * P], F16)

            def edge_phase(table, width, bvec, accw, sstage, evac, block_done=None):
                """psum_t = b*sqrtdeg + table_t^T + A^T table, per dst tile.

                evac(t, psum_tile) consumes the finished accumulation."""
                for bi, blk in enumerate(blocks):
                    psums = {}
                    mm_total = {}
                    mm_done = {}
                    for t in blk:
                        mm_total[t] = 2  # bias + self-loop
                    for g in range(NG):
                        for ent in calls[(bi, g)]["cov"]:
                            for t, lo, hi in ent:
                                mm_total[t] += 1

                    def acc_mm(t, lhsT, rhs):
                        pa = psums[t]
                        k = mm_done[t]
                        nc.tensor.matmul(
                            pa, lhsT, rhs,
                            start=(k == 0), stop=(k == mm_total[t] - 1),
                        )
                        mm_done[t] = k + 1

                    for ti, t in enumerate(blk):
                        pa_t = ps.tile(
                            [accw, P], F32, name=f"acc{ti}", tag=f"acc{ti}"
                        )
                        psums[t] = pa_t[:]
                        mm_done[t] = 0
                        # bias * sqrtdeg row
                        acc_mm(t, bvec[:], sqrow[0:1, t * P : (t + 1) * P])
                        # dense self-loop: psum += table_t^T
                        acc_mm(t, sstage[:, t, 0:width], ident16[:, :])

                    for g in range(NG):
                        info = calls[(bi, g)]
                        B = info["B"]
                        if B == 0:
                            continue
                        sc = len(info["cov"])
                        gbase = g * GSZ
                        gsz = min(GSZ, NPAD - gbase)
                        gi = eb.tile([P, SCMAX * 8], I16, tag="gi")
                        nc.sync.dma_start(
                            gi[:, 0 : B // 16],
                            gidx_in[:, info["off16"] : info["off16"] + B // 16],
                        )
                        msgs = eb.tile([P, SCMAX, P], F16, tag="msgs")
                        nc.gpsimd.dma_gather(
                            msgs[:, 0:sc, :],
                            table[gbase : gbase + gsz, :],
                            gi[:, 0 : B // 16],
                            B,
                            B,
                            P,
                            single_packet=False,
                        )
                        ind = eb.tile([P, SCMAX, P], F16, tag="ind")
                        oc = info["offch"]
                        nc.vector.tensor_tensor(
                            out=ind[:, 0:sc, :],
                            in0=iota16[:, :]
                            .rearrange("p (s d) -> p s d", s=1)
                            .to_broadcast([P, sc, P]),
                            in1=dloc[:, oc : oc + sc]
                            .rearrange("p (s o) -> p s o", o=1)
                            .to_broadcast([P, sc, P]),
                            op=mybir.AluOpType.is_equal,
                        )
                        for k, entries in enumerate(info["cov"]):
                            for t, lo, hi in entries:
                                acc_mm(
                                    t,
                                    msgs[lo:hi, k, 0:width],
                                    ind[lo:hi, k, :],
                                )
                    for ti, t in enumerate(blk):
                        assert mm_done[t] == mm_total[t], (bi, t)
                        evac(t, psums[t])
                    if block_done is not None:
                        block_done(blk)
                    del psums

            # ---- Phase D: layer-1 edges + relu + layer-2 transform ----
            def gsh_blk(blk):
                t0, t1 = blk[0], blk[-1] + 1
                nc.sync.dma_start(
                    gsh.rearrange("(t p) d -> p t d", p=P)[:, t0:t1, :],
                    stage2[:, t0:t1, :],
                )

            def evac1(t, pa):
                nc.scalar.activation(
                    rt16[:, t * P : (t + 1) * P],
                    pa,
                    mybir.ActivationFunctionType.Relu,
                )
                pg = pst2.tile([P, C], F32, tag="ptx")
                nc.tensor.matmul(
                    pg[:], rt16[:, t * P : (t + 1) * P], w2[:], start=True, stop=True
                )
                nc.scalar.activation(
                    stage2[:, t, 0:C],
                    pg[:],
                    mybir.ActivationFunctionType.Copy,
                    scale=dinv2_nm[:, t : t + 1],
                )

            edge_phase(hfull, H, b1, H, stage, evac1, gsh_blk)

            # ---- Phase E: AllGather layer-2 table ----
            nc.gpsimd.collective_compute(
                "AllGather",
                mybir.AluOpType.bypass,
                ins=[gsh[:]],
                outs=[gfull40[:]],
                replica_groups=rgroups,
            )
            for gb in range(0, NPAD, 8192):
                ge = min(gb + 8192, NPAD)
                bt = rs.tile([P, 64, C], F16, tag="restride")
                nw = (ge - gb) // P
                nc.scalar.dma_start(
                    bt[:, 0:nw, :],
                    gfull40[gb:ge, :].rearrange("(c p) e -> p c e", p=P),
                )
                nc.scalar.dma_start(
                    gfull[gb:ge, 0:C].rearrange("(c p) e -> p c e", p=P),
                    bt[:, 0:nw, :],
                )

            # ---- Phase F: layer-2 edges + final transpose/scale ----
            out_stage = stg.tile([P, T, C], F32, tag="ostage")

            def evac2(t, pa):
                sb = io.tile([C, P], F32, tag="ev2")
                nc.vector.tensor_copy(sb[:], pa)
                ptr = pst2.tile([P, C], F32, tag="ptx")
                nc.tensor.transpose(
                    out=ptr[:], in_=sb[:], identity=ident[0:C, 0:C]
                )
                nc.scalar.activation(
                    out_stage[:, t, :],
                    ptr[:],
                    mybir.ActivationFunctionType.Copy,
                    scale=dinv_nm[:, t : t + 1],
                )

            edge_phase(gfull, C, b2, C, stage2, evac2)
            nc.sync.dma_start(out_ext.rearrange("(t p) c -> p t c", p=P)[:], out_stage[:])

    nc.compile()
    return nc


@functools.lru_cache(maxsize=8)
def _build_cached(cfg_key):
    Fdim, H, C, NS, ecnt_t = cfg_key
    return _build((Fdim, H, C, NS, [list(g) for g in ecnt_t]))


# ----------------------------------------------------------------------------
# Host-side sharding / metadata prep
# ----------------------------------------------------------------------------


def _prep(x, edge_index, W1, b1, W2, b2):
    N, Fdim = x.shape
    H = W1.shape[1]
    C = W2.shape[1]
    NS = _round_up(-(-N // NCORE), P)
    T = NS // P
    NPAD = NCORE * NS
    NG = -(-NPAD // GSZ)

    src = np.asarray(edge_index[0], dtype=np.int64)
    dst = np.asarray(edge_index[1], dtype=np.int64)

    deg = np.bincount(dst, minlength=N).astype(np.float32) + 1.0  # + self loop
    deg_pad = np.ones(NPAD, dtype=np.float32)
    deg_pad[:N] = deg

    core = dst // NS
    t_of = (dst % NS) >> 7
    g_of = src >> GSHIFT
    d_of = dst & (P - 1)

    seg_id = (core * NG + g_of) * T + t_of
    cnt = np.bincount(seg_id, minlength=NCORE * NG * T).reshape(NCORE, NG, T)
    ecnt = _round_up(cnt.max(axis=0), 64)  # [NG, T] shared; 64: legal PE base partitions are 0,32,64

    blocks = [list(range(b, min(b + TBSZ, T))) for b in range(0, T, TBSZ)]
    ecnt_l = [[int(v) for v in row] for row in ecnt]
    calls, E_IDX, NCH = _call_layout(ecnt_l, blocks, NG)

    # global idx-stream position of each (g,t) segment
    seg_base = np.zeros((NG, T), dtype=np.int64)
    for bi, blk in enumerate(blocks):
        for g in range(NG):
            info = calls[(bi, g)]
            s = 0
            for t in blk:
                seg_base[g, t] = info["off16"] * 16 + s
                s += ecnt[g, t]

    # rank of each edge inside its (core,g,t) segment
    order = np.argsort(seg_id, kind="stable")
    seg_sorted = seg_id[order]
    starts = np.searchsorted(seg_sorted, np.arange(NCORE * NG * T))
    rank = np.arange(len(order)) - starts[seg_sorted]
    g_sorted = (seg_sorted // T) % NG
    t_sorted = seg_sorted % T
    pos_sorted = seg_base[g_sorted, t_sorted] + rank
    core_sorted = seg_sorted // (NG * T)

    gidx_all = np.zeros((NCORE, E_IDX), dtype=np.int16)
    dloc_all = np.full((NCORE, NCH * P), -1.0, dtype=np.float16)
    # map idx-stream position -> chunk-grid position (call-aligned)
    chunk_pos = np.zeros(max(E_IDX, 1), dtype=np.int64)
    for bi in range(len(blocks)):
        for g in range(NG):
            info = calls[(bi, g)]
            B = info["B"]
            if B == 0:
                continue
            a = info["off16"] * 16
            chunk_pos[a : a + B] = info["offch"] * P + np.arange(B)

    gidx_all[core_sorted, pos_sorted] = (
        src[order] - (g_of[order] << GSHIFT)
    ).astype(np.int16)
    dloc_all[core_sorted, chunk_pos[pos_sorted]] = d_of[order].astype(np.float16)

    x_pad = np.zeros((NPAD, Fdim), dtype=np.float32)
    x_pad[:N] = np.asarray(x, dtype=np.float32)

    in_maps = []
    for c in range(NCORE):
        xT = np.ascontiguousarray(x_pad[c * NS : (c + 1) * NS].T)
        dshard = deg_pad[c * NS : (c + 1) * NS]
        deg_nm = np.ascontiguousarray(dshard.reshape(T, P).T)
        deg_row = dshard.reshape(1, NS)
        flat = gidx_all[c]
        gidx_w = np.tile(
            np.ascontiguousarray(flat.reshape(E_IDX // 16, 16).T), (NCORE, 1)
        )
        dloc_w = np.ascontiguousarray(dloc_all[c].reshape(NCH, P).T)
        in_maps.append(
            {
                "xT": xT,
                "deg_nm": deg_nm,
                "deg_row": deg_row,
                "W1": np.asarray(W1, dtype=np.float32).reshape(Fdim, H),
                "W2": np.asarray(W2, dtype=np.float32).reshape(H, C),
                "b1": np.asarray(b1, dtype=np.float32).reshape(1, H),
                "b2": np.asarray(b2, dtype=np.float32).reshape(1, C),
                "gidx": gidx_w,
                "dloc": dloc_w,
            }
        )

    cfg_key = (Fdim, H, C, NS, tuple(tuple(int(v) for v in row) for row in ecnt))
    return cfg_key, in_maps, N, NS, C


def _run(x, edge_index, W1, b1, W2, b2, trace=False):
    cfg_key, in_maps, N, NS, C = _prep(x, edge_index, W1, b1, W2, b2)
    nc = _build_cached(cfg_key)
    res = run_bass_kernel_spmd(nc, in_maps, list(range(NCORE)), trace=trace)
    shards = [res.results[c]["out_nm"] for c in range(NCORE)]
    out = np.concatenate(shards, axis=0)[:N]
    return np.ascontiguousarray(out, dtype=np.float32), res


def kernel(x, edge_index, W1, b1, W2, b2):
    out, _ = _run(x, edge_index, W1, b1, W2, b2)
    return out


# revision 23
# speedup vs baseline: 1.1525x; 1.1525x over previous
"""GCN (2-layer, PyG GCNConv-style) on 8 Trainium2 NeuronCores.

Strategy (sharding_hint): nodes sharded across the 8 cores (data parallel on
the node dim); edges partitioned by destination core so the scatter-add stays
local; per layer the dinv-scaled transformed features are AllGathered so each
core can gather arbitrary source rows; weights replicated.

Math (per layer, A' = A + I, dinv = deg^-1/2):
    out = dinv . (A'^T (dinv . (x @ W))) + b
We fold norms so no per-edge scaling is needed:
  - table  = dinv . (x @ W)                    (per-node scale, ACT)
  - agg    = A^T table + table + b * sqrtdeg   (PE one-hot matmuls; the
             self-loop term "+ table" is a dense per-tile transpose matmul,
             so self loops never enter the gathered edge streams)
  - layer1 h2 = dinv . relu(agg)               (dinv moved past relu, dinv>0)
    the dinv is then folded into layer2's table scale (dinv^2).
  - layer2 out = dinv . agg2                   (final per-node scale)

Edge aggregation: edges are grouped on host by (dst-tile t of 128 nodes,
src-group g of 32768 nodes); per (g,t) the segment is padded to a multiple of
16 only (pad idx=0, pad dst label -1).  Segments are concatenated per
(g, tile-block) into one gather call; matmul chunks of 128 gathered rows may
straddle segment boundaries, which is handled with partition-sliced one-hot
matmuls (the straddle map is static because segment sizes are shared across
cores; per-core count variation hides in the dst labels, where -1 kills the
one-hot).  Scatter-add accumulates directly into one PSUM tile per dst tile.
"""

import functools
import numpy as np

import concourse.bacc as bacc
import concourse.mybir as mybir
import concourse.tile as tile
from concourse.bass_utils import run_bass_kernel_spmd
from concourse.masks import make_identity

NCORE = 8
P = 128
GSHIFT = 15  # src-group size 32768 (int16 index range)
GSZ = 1 << GSHIFT
TBSZ = 4  # dst tiles per gather block

F16 = mybir.dt.float16
F32 = mybir.dt.float32
I16 = mybir.dt.int16


def _round_up(a, b):
    return (a + b - 1) // b * b


def _call_layout(ecnt, blocks, NG):
    """Static stream layout.  Returns per-call info:
    calls[(bi, g)] = dict(B, off16, offch, cov) where cov[k] = list of
    (t, lo, hi) partition-slices of chunk k."""
    calls = {}
    off16 = 0  # global gidx offset, units of 16 idxs
    offch = 0  # global chunk offset (dloc columns)
    for bi, blk in enumerate(blocks):
        for g in range(NG):
            segs = []
            s = 0
            for t in blk:
                e = ecnt[g][t]
                segs.append((t, s, e))
                s += e
            B = s
            K = -(-B // P) if B else 0
            cov = []
            for k in range(K):
                lo_k, hi_k = k * P, (k + 1) * P
                entries = []
                for t, st, e in segs:
                    if e == 0:
                        continue
                    a, b = max(st, lo_k), min(st + e, hi_k)
                    if a < b:
                        entries.append([t, a - lo_k, b - lo_k])
                cov.append(entries)
            calls[(bi, g)] = dict(B=B, off16=off16, offch=offch, cov=cov)
            off16 += B // 16
            offch += K
    return calls, off16 * 16, offch


# ----------------------------------------------------------------------------
# Bass program (identical for all 8 cores; per-core data differs via inputs)
# ----------------------------------------------------------------------------


def _build(cfg):
    """cfg = (F, H, C, NS, ecnt) with ecnt[g][t] = padded16 edge count."""
    Fdim, H, C, NS, ecnt = cfg
    ecnt = [list(gr) for gr in ecnt]
    T = NS // P
    NPAD = NCORE * NS
    NG = len(ecnt)
    blocks = [list(range(b, min(b + TBSZ, T))) for b in range(0, T, TBSZ)]
    calls, E_IDX, NCH = _call_layout(ecnt, blocks, NG)
    SCMAX = max(len(c["cov"]) for c in calls.values())

    nc = bacc.Bacc(None, target_bir_lowering=False)

    # ---- I/O ----
    xT_in = nc.dram_tensor("xT", [P, NS], F32, kind="ExternalInput")
    degnm_in = nc.dram_tensor("deg_nm", [P, T], F32, kind="ExternalInput")
    degrow_in = nc.dram_tensor("deg_row", [1, NS], F32, kind="ExternalInput")
    w1_in = nc.dram_tensor("W1", [Fdim, H], F32, kind="ExternalInput")
    w2_in = nc.dram_tensor("W2", [H, C], F32, kind="ExternalInput")
    b1_in = nc.dram_tensor("b1", [1, H], F32, kind="ExternalInput")
    b2_in = nc.dram_tensor("b2", [1, C], F32, kind="ExternalInput")
    gidx_in = nc.dram_tensor("gidx", [P, E_IDX // 16], I16, kind="ExternalInput")
    dloc_in = nc.dram_tensor("dloc", [P, NCH], F16, kind="ExternalInput")
    out_ext = nc.dram_tensor("out_nm", [NS, C], F32, kind="ExternalOutput")

    hsh = nc.dram_tensor("hsh", [NS, P], F16)
    gsh = nc.dram_tensor("gsh", [NS, P], F16)
    hfull = nc.dram_tensor("hfull", [NPAD, P], F16, addr_space="Shared")
    gfull = nc.dram_tensor("gfull", [NPAD, P], F16, addr_space="Shared")
    rgroups = [list(range(NCORE))]

    with tile.TileContext(nc) as tc:
        with (
            tc.tile_pool(name="con", bufs=1) as con,
            tc.tile_pool(name="meta", bufs=1) as meta,
            tc.tile_pool(name="stg", bufs=1) as stg,
            tc.tile_pool(name="io", bufs=3) as io,
            tc.tile_pool(name="eb", bufs=3) as eb,
            tc.tile_pool(name="ps", bufs=1, space="PSUM") as ps,
            tc.tile_pool(name="pst", bufs=2, space="PSUM") as pst,
            tc.tile_pool(name="pst2", bufs=2, space="PSUM") as pst2,
        ):
            # ---- Phase A: constants / metadata ----
            dloc = meta.tile([P, NCH], F16)
            nc.sync.dma_start(dloc[:], dloc_in[:])

            w1f = con.tile([Fdim, H], F32)
            nc.sync.dma_start(w1f[:], w1_in[:])
            w1 = con.tile([Fdim, H], F16)
            nc.vector.tensor_copy(w1[:], w1f[:])
            w2f = con.tile([H, C], F32)
            nc.sync.dma_start(w2f[:], w2_in[:])
            w2 = con.tile([H, C], F16)
            nc.vector.tensor_copy(w2[:], w2f[:])
            b1f = con.tile([1, H], F32)
            nc.sync.dma_start(b1f[:], b1_in[:])
            b1 = con.tile([1, H], F16)
            nc.vector.tensor_copy(b1[:], b1f[:])
            b2f = con.tile([1, C], F32)
            nc.sync.dma_start(b2f[:], b2_in[:])
            b2 = con.tile([1, C], F16)
            nc.vector.tensor_copy(b2[:], b2f[:])

            degnm = con.tile([P, T], F32)
            nc.sync.dma_start(degnm[:], degnm_in[:])
            sq_nm = con.tile([P, T], F32)
            nc.scalar.activation(sq_nm[:], degnm[:], mybir.ActivationFunctionType.Sqrt)
            dinv_nm = con.tile([P, T], F32)
            nc.vector.reciprocal(dinv_nm[:], sq_nm[:])
            dinv2_nm = con.tile([P, T], F32)
            nc.vector.tensor_mul(dinv2_nm[:], dinv_nm[:], dinv_nm[:])

            degrow = con.tile([1, NS], F32)
            nc.sync.dma_start(degrow[:], degrow_in[:])
            sqrow = con.tile([1, NS], F16)
            nc.scalar.activation(sqrow[:], degrow[:], mybir.ActivationFunctionType.Sqrt)

            iota_i = con.tile([P, P], I16)
            nc.gpsimd.iota(iota_i[:], pattern=[[1, P]], base=0, channel_multiplier=0)
            iota16 = con.tile([P, P], F16)
            nc.vector.tensor_copy(iota16[:], iota_i[:])

            ident = con.tile([P, P], F32)
            make_identity(nc, ident[:])
            ident16 = con.tile([P, P], F16)
            nc.vector.tensor_copy(ident16[:], ident[:])

            stage = stg.tile([P, T, H], F16, tag="stage")
            stage2 = stg.tile([P, T, C], F16, tag="stage2")

            # ---- Phase B: layer-1 transform, build h' table ----
            for t in range(T):
                xt = io.tile([P, P], F32, tag="xt")
                nc.sync.dma_start(xt[:], xT_in[:, t * P : (t + 1) * P])
                xt16 = io.tile([P, P], F16, tag="xt16")
                nc.vector.tensor_copy(xt16[:], xt[:])
                ph = pst.tile([P, H], F32, tag="pt")
                nc.tensor.matmul(ph[:], xt16[:], w1[:], start=True, stop=True)
                nc.scalar.activation(
                    stage[:, t, 0:H],
                    ph[:],
                    mybir.ActivationFunctionType.Copy,
                    scale=dinv_nm[:, t : t + 1],
                )
            nc.sync.dma_start(
                hsh.rearrange("(t p) d -> p t d", p=P)[:, :, 0:H], stage[:]
            )

            # ---- Phase C: AllGather layer-1 table ----
            nc.gpsimd.collective_compute(
                "AllGather",
                mybir.AluOpType.bypass,
                ins=[hsh[:]],
                outs=[hfull[:]],
                replica_groups=rgroups,
            )

            rt16 = stg.tile([H, T * P], F16)

            def edge_phase(table, width, bvec, accw, sstage, evac, block_done=None):
                """psum_t = b*sqrtdeg + table_t^T + A^T table, per dst tile.

                evac(t, psum_tile) consumes the finished accumulation."""
                for bi, blk in enumerate(blocks):
                    psums = {}
                    mm_total = {}
                    mm_done = {}
                    for t in blk:
                        mm_total[t] = 2  # bias + self-loop
                    for g in range(NG):
                        for ent in calls[(bi, g)]["cov"]:
                            for t, lo, hi in ent:
                                mm_total[t] += 1

                    def acc_mm(t, lhsT, rhs):
                        pa = psums[t]
                        k = mm_done[t]
                        nc.tensor.matmul(
                            pa, lhsT, rhs,
                            start=(k == 0), stop=(k == mm_total[t] - 1),
                        )
                        mm_done[t] = k + 1

                    for ti, t in enumerate(blk):
                        pa_t = ps.tile(
                            [accw, P], F32, name=f"acc{ti}", tag=f"acc{ti}"
                        )
                        psums[t] = pa_t[:]
                        mm_done[t] = 0
                        # bias * sqrtdeg row
                        acc_mm(t, bvec[:], sqrow[0:1, t * P : (t + 1) * P])
                        # dense self-loop: psum += table_t^T
                        acc_mm(t, sstage[:, t, 0:width], ident16[:, :])

                    for g in range(NG):
                        info = calls[(bi, g)]
                        B = info["B"]
                        if B == 0:
                            continue
                        sc = len(info["cov"])
                        gbase = g * GSZ
                        gsz = min(GSZ, NPAD - gbase)
                        gi = eb.tile([P, SCMAX * 8], I16, tag="gi")
                        nc.sync.dma_start(
                            gi[:, 0 : B // 16],
                            gidx_in[:, info["off16"] : info["off16"] + B // 16],
                        )
                        msgs = eb.tile([P, SCMAX, P], F16, tag="msgs")
                        nc.gpsimd.dma_gather(
                            msgs[:, 0:sc, :],
                            table[gbase : gbase + gsz, :],
                            gi[:, 0 : B // 16],
                            B,
                            B,
                            P,
                            single_packet=False,
                        )
                        ind = eb.tile([P, SCMAX, P], F16, tag="ind")
                        oc = info["offch"]
                        nc.vector.tensor_tensor(
                            out=ind[:, 0:sc, :],
                            in0=iota16[:, :]
                            .rearrange("p (s d) -> p s d", s=1)
                            .to_broadcast([P, sc, P]),
                            in1=dloc[:, oc : oc + sc]
                            .rearrange("p (s o) -> p s o", o=1)
                            .to_broadcast([P, sc, P]),
                            op=mybir.AluOpType.is_equal,
                        )
                        for k, entries in enumerate(info["cov"]):
                            for t, lo, hi in entries:
                                acc_mm(
                                    t,
                                    msgs[lo:hi, k, 0:width],
                                    ind[lo:hi, k, :],
                                )
                    for ti, t in enumerate(blk):
                        assert mm_done[t] == mm_total[t], (bi, t)
                        evac(t, psums[t])
                    if block_done is not None:
                        block_done(blk)
                    del psums

            # ---- Phase D: layer-1 edges + relu + layer-2 transform ----
            def gsh_blk(blk):
                t0, t1 = blk[0], blk[-1] + 1
                nc.sync.dma_start(
                    gsh.rearrange("(t p) d -> p t d", p=P)[:, t0:t1, 0:C],
                    stage2[:, t0:t1, :],
                )

            def evac1(t, pa):
                nc.scalar.activation(
                    rt16[:, t * P : (t + 1) * P],
                    pa,
                    mybir.ActivationFunctionType.Relu,
                )
                pg = pst2.tile([P, C], F32, tag="ptx")
                nc.tensor.matmul(
                    pg[:], rt16[:, t * P : (t + 1) * P], w2[:], start=True, stop=True
                )
                nc.scalar.activation(
                    stage2[:, t, 0:C],
                    pg[:],
                    mybir.ActivationFunctionType.Copy,
                    scale=dinv2_nm[:, t : t + 1],
                )

            edge_phase(hfull, H, b1, H, stage, evac1, gsh_blk)

            # ---- Phase E: AllGather layer-2 table ----
            nc.gpsimd.collective_compute(
                "AllGather",
                mybir.AluOpType.bypass,
                ins=[gsh[:]],
                outs=[gfull[:]],
                replica_groups=rgroups,
            )

            # ---- Phase F: layer-2 edges + final transpose/scale ----
            out_stage = stg.tile([P, T, C], F32, tag="ostage")

            def evac2(t, pa):
                sb = io.tile([C, P], F32, tag="ev2")
                nc.vector.tensor_copy(sb[:], pa)
                ptr = pst2.tile([P, C], F32, tag="ptx")
                nc.tensor.transpose(
                    out=ptr[:], in_=sb[:], identity=ident[0:C, 0:C]
                )
                nc.scalar.activation(
                    out_stage[:, t, :],
                    ptr[:],
                    mybir.ActivationFunctionType.Copy,
                    scale=dinv_nm[:, t : t + 1],
                )

            edge_phase(gfull, C, b2, C, stage2, evac2)
            nc.sync.dma_start(out_ext.rearrange("(t p) c -> p t c", p=P)[:], out_stage[:])

    nc.compile()
    return nc


@functools.lru_cache(maxsize=8)
def _build_cached(cfg_key):
    Fdim, H, C, NS, ecnt_t = cfg_key
    return _build((Fdim, H, C, NS, [list(g) for g in ecnt_t]))


# ----------------------------------------------------------------------------
# Host-side sharding / metadata prep
# ----------------------------------------------------------------------------


def _prep(x, edge_index, W1, b1, W2, b2):
    N, Fdim = x.shape
    H = W1.shape[1]
    C = W2.shape[1]
    NS = _round_up(-(-N // NCORE), P)
    T = NS // P
    NPAD = NCORE * NS
    NG = -(-NPAD // GSZ)

    src = np.asarray(edge_index[0], dtype=np.int64)
    dst = np.asarray(edge_index[1], dtype=np.int64)

    deg = np.bincount(dst, minlength=N).astype(np.float32) + 1.0  # + self loop
    deg_pad = np.ones(NPAD, dtype=np.float32)
    deg_pad[:N] = deg

    core = dst // NS
    t_of = (dst % NS) >> 7
    g_of = src >> GSHIFT
    d_of = dst & (P - 1)

    seg_id = (core * NG + g_of) * T + t_of
    cnt = np.bincount(seg_id, minlength=NCORE * NG * T).reshape(NCORE, NG, T)
    ecnt = _round_up(cnt.max(axis=0), 64)  # [NG, T] shared; 64: legal PE base partitions are 0,32,64

    blocks = [list(range(b, min(b + TBSZ, T))) for b in range(0, T, TBSZ)]
    ecnt_l = [[int(v) for v in row] for row in ecnt]
    calls, E_IDX, NCH = _call_layout(ecnt_l, blocks, NG)

    # global idx-stream position of each (g,t) segment
    seg_base = np.zeros((NG, T), dtype=np.int64)
    for bi, blk in enumerate(blocks):
        for g in range(NG):
            info = calls[(bi, g)]
            s = 0
            for t in blk:
                seg_base[g, t] = info["off16"] * 16 + s
                s += ecnt[g, t]

    # rank of each edge inside its (core,g,t) segment
    order = np.argsort(seg_id, kind="stable")
    seg_sorted = seg_id[order]
    starts = np.searchsorted(seg_sorted, np.arange(NCORE * NG * T))
    rank = np.arange(len(order)) - starts[seg_sorted]
    g_sorted = (seg_sorted // T) % NG
    t_sorted = seg_sorted % T
    pos_sorted = seg_base[g_sorted, t_sorted] + rank
    core_sorted = seg_sorted // (NG * T)

    gidx_all = np.zeros((NCORE, E_IDX), dtype=np.int16)
    dloc_all = np.full((NCORE, NCH * P), -1.0, dtype=np.float16)
    # map idx-stream position -> chunk-grid position (call-aligned)
    chunk_pos = np.zeros(max(E_IDX, 1), dtype=np.int64)
    for bi in range(len(blocks)):
        for g in range(NG):
            info = calls[(bi, g)]
            B = info["B"]
            if B == 0:
                continue
            a = info["off16"] * 16
            chunk_pos[a : a + B] = info["offch"] * P + np.arange(B)

    gidx_all[core_sorted, pos_sorted] = (
        src[order] - (g_of[order] << GSHIFT)
    ).astype(np.int16)
    dloc_all[core_sorted, chunk_pos[pos_sorted]] = d_of[order].astype(np.float16)

    x_pad = np.zeros((NPAD, Fdim), dtype=np.float32)
    x_pad[:N] = np.asarray(x, dtype=np.float32)

    in_maps = []
    for c in range(NCORE):
        xT = np.ascontiguousarray(x_pad[c * NS : (c + 1) * NS].T)
        dshard = deg_pad[c * NS : (c + 1) * NS]
        deg_nm = np.ascontiguousarray(dshard.reshape(T, P).T)
        deg_row = dshard.reshape(1, NS)
        flat = gidx_all[c]
        gidx_w = np.tile(
            np.ascontiguousarray(flat.reshape(E_IDX // 16, 16).T), (NCORE, 1)
        )
        dloc_w = np.ascontiguousarray(dloc_all[c].reshape(NCH, P).T)
        in_maps.append(
            {
                "xT": xT,
                "deg_nm": deg_nm,
                "deg_row": deg_row,
                "W1": np.asarray(W1, dtype=np.float32).reshape(Fdim, H),
                "W2": np.asarray(W2, dtype=np.float32).reshape(H, C),
                "b1": np.asarray(b1, dtype=np.float32).reshape(1, H),
                "b2": np.asarray(b2, dtype=np.float32).reshape(1, C),
                "gidx": gidx_w,
                "dloc": dloc_w,
            }
        )

    cfg_key = (Fdim, H, C, NS, tuple(tuple(int(v) for v in row) for row in ecnt))
    return cfg_key, in_maps, N, NS, C


def _run(x, edge_index, W1, b1, W2, b2, trace=False):
    cfg_key, in_maps, N, NS, C = _prep(x, edge_index, W1, b1, W2, b2)
    nc = _build_cached(cfg_key)
    res = run_bass_kernel_spmd(nc, in_maps, list(range(NCORE)), trace=trace)
    shards = [res.results[c]["out_nm"] for c in range(NCORE)]
    out = np.concatenate(shards, axis=0)[:N]
    return np.ascontiguousarray(out, dtype=np.float32), res


def kernel(x, edge_index, W1, b1, W2, b2):
    out, _ = _run(x, edge_index, W1, b1, W2, b2)
    return out


# revision 24
# speedup vs baseline: 1.1711x; 1.0161x over previous
"""GCN (2-layer, PyG GCNConv-style) on 8 Trainium2 NeuronCores.

Strategy (sharding_hint): nodes sharded across the 8 cores (data parallel on
the node dim); edges partitioned by destination core so the scatter-add stays
local; per layer the dinv-scaled transformed features are AllGathered so each
core can gather arbitrary source rows; weights replicated.

Math (per layer, A' = A + I, dinv = deg^-1/2):
    out = dinv . (A'^T (dinv . (x @ W))) + b
We fold norms so no per-edge scaling is needed:
  - table  = dinv . (x @ W)                    (per-node scale, ACT)
  - agg    = A^T table + table + b * sqrtdeg   (PE one-hot matmuls; the
             self-loop term "+ table" is a dense per-tile transpose matmul,
             so self loops never enter the gathered edge streams)
  - layer1 h2 = dinv . relu(agg)               (dinv moved past relu, dinv>0)
    the dinv is then folded into layer2's table scale (dinv^2).
  - layer2 out = dinv . agg2                   (final per-node scale)

Edge aggregation: edges are grouped on host by (dst-tile t of 128 nodes,
src-group g of 32768 nodes); per (g,t) the segment is padded to a multiple of
16 only (pad idx=0, pad dst label -1).  Segments are concatenated per
(g, tile-block) into one gather call; matmul chunks of 128 gathered rows may
straddle segment boundaries, which is handled with partition-sliced one-hot
matmuls (the straddle map is static because segment sizes are shared across
cores; per-core count variation hides in the dst labels, where -1 kills the
one-hot).  Scatter-add accumulates directly into one PSUM tile per dst tile.
"""

import functools
import numpy as np

import concourse.bacc as bacc
import concourse.mybir as mybir
import concourse.tile as tile
from concourse.bass_utils import run_bass_kernel_spmd
from concourse.masks import make_identity

NCORE = 8
P = 128
GSHIFT = 15  # src-group size 32768 (int16 index range)
GSZ = 1 << GSHIFT
TBSZ = 4  # dst tiles per gather block

F16 = mybir.dt.float16
F32 = mybir.dt.float32
I16 = mybir.dt.int16


def _round_up(a, b):
    return (a + b - 1) // b * b


def _call_layout(ecnt, blocks, NG):
    """Static stream layout.  Returns per-call info:
    calls[(bi, g)] = dict(B, off16, offch, cov) where cov[k] = list of
    (t, lo, hi) partition-slices of chunk k."""
    calls = {}
    off16 = 0  # global gidx offset, units of 16 idxs
    offch = 0  # global chunk offset (dloc columns)
    for bi, blk in enumerate(blocks):
        for g in range(NG):
            segs = []
            s = 0
            for t in blk:
                e = ecnt[g][t]
                segs.append((t, s, e))
                s += e
            B = s
            K = -(-B // P) if B else 0
            cov = []
            for k in range(K):
                lo_k, hi_k = k * P, (k + 1) * P
                entries = []
                for t, st, e in segs:
                    if e == 0:
                        continue
                    a, b = max(st, lo_k), min(st + e, hi_k)
                    if a < b:
                        entries.append([t, a - lo_k, b - lo_k])
                cov.append(entries)
            calls[(bi, g)] = dict(B=B, off16=off16, offch=offch, cov=cov)
            off16 += B // 16
            offch += K
    return calls, off16 * 16, offch


# ----------------------------------------------------------------------------
# Bass program (identical for all 8 cores; per-core data differs via inputs)
# ----------------------------------------------------------------------------


def _build(cfg):
    """cfg = (F, H, C, NS, ecnt) with ecnt[g][t] = padded16 edge count."""
    Fdim, H, C, NS, ecnt = cfg
    ecnt = [list(gr) for gr in ecnt]
    T = NS // P
    NPAD = NCORE * NS
    NG = len(ecnt)
    blocks = [list(range(b, min(b + TBSZ, T))) for b in range(0, T, TBSZ)]
    calls, E_IDX, NCH = _call_layout(ecnt, blocks, NG)
    SCMAX = max(len(c["cov"]) for c in calls.values())

    nc = bacc.Bacc(None, target_bir_lowering=False)

    # ---- I/O ----
    xT_in = nc.dram_tensor("xT", [P, NS], F32, kind="ExternalInput")
    degnm_in = nc.dram_tensor("deg_nm", [P, T], F32, kind="ExternalInput")
    degrow_in = nc.dram_tensor("deg_row", [1, NS], F32, kind="ExternalInput")
    w1_in = nc.dram_tensor("W1", [Fdim, H], F32, kind="ExternalInput")
    w2_in = nc.dram_tensor("W2", [H, C], F32, kind="ExternalInput")
    b1_in = nc.dram_tensor("b1", [1, H], F32, kind="ExternalInput")
    b2_in = nc.dram_tensor("b2", [1, C], F32, kind="ExternalInput")
    gidx_in = nc.dram_tensor("gidx", [P, E_IDX // 16], I16, kind="ExternalInput")
    dloc_in = nc.dram_tensor("dloc", [P, NCH], F16, kind="ExternalInput")
    out_ext = nc.dram_tensor("out_nm", [NS, C], F32, kind="ExternalOutput")

    hsh = nc.dram_tensor("hsh", [NS, P], F16)
    gsh = nc.dram_tensor("gsh", [NS, P], F16)
    hfull = nc.dram_tensor("hfull", [NPAD, P], F16, addr_space="Shared")
    gfull = nc.dram_tensor("gfull", [NPAD, P], F16, addr_space="Shared")
    rgroups = [list(range(NCORE))]

    with tile.TileContext(nc) as tc:
        with (
            tc.tile_pool(name="con", bufs=1) as con,
            tc.tile_pool(name="meta", bufs=1) as meta,
            tc.tile_pool(name="stg", bufs=1) as stg,
            tc.tile_pool(name="io", bufs=3) as io,
            tc.tile_pool(name="eb", bufs=3) as eb,
            tc.tile_pool(name="ps", bufs=1, space="PSUM") as ps,
            tc.tile_pool(name="pst", bufs=2, space="PSUM") as pst,
            tc.tile_pool(name="pst2", bufs=2, space="PSUM") as pst2,
        ):
            # ---- Phase A: constants / metadata ----
            dloc = meta.tile([P, NCH], F16)
            nc.sync.dma_start(dloc[:], dloc_in[:])

            w1f = con.tile([Fdim, H], F32)
            nc.sync.dma_start(w1f[:], w1_in[:])
            w1 = con.tile([Fdim, H], F16)
            nc.vector.tensor_copy(w1[:], w1f[:])
            w2f = con.tile([H, C], F32)
            nc.sync.dma_start(w2f[:], w2_in[:])
            w2 = con.tile([H, C], F16)
            nc.vector.tensor_copy(w2[:], w2f[:])
            b1f = con.tile([1, H], F32)
            nc.sync.dma_start(b1f[:], b1_in[:])
            b1 = con.tile([1, H], F16)
            nc.vector.tensor_copy(b1[:], b1f[:])
            b2f = con.tile([1, C], F32)
            nc.sync.dma_start(b2f[:], b2_in[:])
            b2 = con.tile([1, C], F16)
            nc.vector.tensor_copy(b2[:], b2f[:])

            degnm = con.tile([P, T], F32)
            nc.sync.dma_start(degnm[:], degnm_in[:])
            sq_nm = con.tile([P, T], F32)
            nc.scalar.activation(sq_nm[:], degnm[:], mybir.ActivationFunctionType.Sqrt)
            dinv_nm = con.tile([P, T], F32)
            nc.vector.reciprocal(dinv_nm[:], sq_nm[:])
            dinv2_nm = con.tile([P, T], F32)
            nc.vector.tensor_mul(dinv2_nm[:], dinv_nm[:], dinv_nm[:])

            degrow = con.tile([1, NS], F32)
            nc.sync.dma_start(degrow[:], degrow_in[:])
            sqrow = con.tile([1, NS], F16)
            nc.scalar.activation(sqrow[:], degrow[:], mybir.ActivationFunctionType.Sqrt)

            iota_i = con.tile([P, P], I16)
            nc.gpsimd.iota(iota_i[:], pattern=[[1, P]], base=0, channel_multiplier=0)
            iota16 = con.tile([P, P], F16)
            nc.vector.tensor_copy(iota16[:], iota_i[:])

            ident = con.tile([P, P], F32)
            make_identity(nc, ident[:])
            ident16 = con.tile([P, P], F16)
            nc.vector.tensor_copy(ident16[:], ident[:])

            stage = stg.tile([P, T, H], F16, tag="stage")
            stage2 = stg.tile([P, T, C], F16, tag="stage2")

            # ---- Phase B: layer-1 transform; quarter AllGathers fire as
            # soon as the tiles covering each table quarter are staged ----
            QS, QF = NS // 4, NPAD // 4

            def ag_quarter(sh, full, q):
                nc.gpsimd.collective_compute(
                    "AllGather",
                    mybir.AluOpType.bypass,
                    ins=[sh[q * QS : (q + 1) * QS, :]],
                    outs=[full[q * QF : (q + 1) * QF, :]],
                    replica_groups=rgroups,
                )

            hq = 0
            hw0 = 0
            for t in range(T):
                xt = io.tile([P, P], F32, tag="xt")
                nc.sync.dma_start(xt[:], xT_in[:, t * P : (t + 1) * P])
                xt16 = io.tile([P, P], F16, tag="xt16")
                nc.vector.tensor_copy(xt16[:], xt[:])
                ph = pst.tile([P, H], F32, tag="pt")
                nc.tensor.matmul(ph[:], xt16[:], w1[:], start=True, stop=True)
                nc.scalar.activation(
                    stage[:, t, 0:H],
                    ph[:],
                    mybir.ActivationFunctionType.Copy,
                    scale=dinv_nm[:, t : t + 1],
                )
                while hq < 4 and (t + 1) * P >= (hq + 1) * QS:
                    nc.sync.dma_start(
                        hsh.rearrange("(t p) d -> p t d", p=P)[:, hw0 : t + 1, 0:H],
                        stage[:, hw0 : t + 1, :],
                    )
                    hw0 = t + 1
                    ag_quarter(hsh, hfull, hq)
                    hq += 1

            rt16 = stg.tile([H, T * P], F16)

            def edge_phase(table, width, bvec, accw, sstage, evac, block_done=None):
                """psum_t = b*sqrtdeg + table_t^T + A^T table, per dst tile.

                evac(t, psum_tile) consumes the finished accumulation."""
                for bi, blk in enumerate(blocks):
                    psums = {}
                    mm_total = {}
                    mm_done = {}
                    for t in blk:
                        mm_total[t] = 2  # bias + self-loop
                    for g in range(NG):
                        for ent in calls[(bi, g)]["cov"]:
                            for t, lo, hi in ent:
                                mm_total[t] += 1

                    def acc_mm(t, lhsT, rhs):
                        pa = psums[t]
                        k = mm_done[t]
                        nc.tensor.matmul(
                            pa, lhsT, rhs,
                            start=(k == 0), stop=(k == mm_total[t] - 1),
                        )
                        mm_done[t] = k + 1

                    for ti, t in enumerate(blk):
                        pa_t = ps.tile(
                            [accw, P], F32, name=f"acc{ti}", tag=f"acc{ti}"
                        )
                        psums[t] = pa_t[:]
                        mm_done[t] = 0
                        # bias * sqrtdeg row
                        acc_mm(t, bvec[:], sqrow[0:1, t * P : (t + 1) * P])
                        # dense self-loop: psum += table_t^T
                        acc_mm(t, sstage[:, t, 0:width], ident16[:, :])

                    for g in range(NG):
                        info = calls[(bi, g)]
                        B = info["B"]
                        if B == 0:
                            continue
                        sc = len(info["cov"])
                        gbase = g * GSZ
                        gsz = min(GSZ, NPAD - gbase)
                        gi = eb.tile([P, SCMAX * 8], I16, tag="gi")
                        nc.sync.dma_start(
                            gi[:, 0 : B // 16],
                            gidx_in[:, info["off16"] : info["off16"] + B // 16],
                        )
                        msgs = eb.tile([P, SCMAX, P], F16, tag="msgs")
                        nc.gpsimd.dma_gather(
                            msgs[:, 0:sc, :],
                            table[gbase : gbase + gsz, :],
                            gi[:, 0 : B // 16],
                            B,
                            B,
                            P,
                            single_packet=False,
                        )
                        ind = eb.tile([P, SCMAX, P], F16, tag="ind")
                        oc = info["offch"]
                        nc.vector.tensor_tensor(
                            out=ind[:, 0:sc, :],
                            in0=iota16[:, :]
                            .rearrange("p (s d) -> p s d", s=1)
                            .to_broadcast([P, sc, P]),
                            in1=dloc[:, oc : oc + sc]
                            .rearrange("p (s o) -> p s o", o=1)
                            .to_broadcast([P, sc, P]),
                            op=mybir.AluOpType.is_equal,
                        )
                        for k, entries in enumerate(info["cov"]):
                            for t, lo, hi in entries:
                                acc_mm(
                                    t,
                                    msgs[lo:hi, k, 0:width],
                                    ind[lo:hi, k, :],
                                )
                    for ti, t in enumerate(blk):
                        assert mm_done[t] == mm_total[t], (bi, t)
                        evac(t, psums[t])
                    if block_done is not None:
                        block_done(blk)
                    del psums

            # ---- Phase D: layer-1 edges + relu + layer-2 transform ----
            gq_state = [0]

            def gsh_blk(blk):
                t0, t1 = blk[0], blk[-1] + 1
                nc.sync.dma_start(
                    gsh.rearrange("(t p) d -> p t d", p=P)[:, t0:t1, 0:C],
                    stage2[:, t0:t1, :],
                )
                while gq_state[0] < 4 and t1 * P >= (gq_state[0] + 1) * QS:
                    ag_quarter(gsh, gfull, gq_state[0])
                    gq_state[0] += 1

            def evac1(t, pa):
                nc.scalar.activation(
                    rt16[:, t * P : (t + 1) * P],
                    pa,
                    mybir.ActivationFunctionType.Relu,
                )
                pg = pst2.tile([P, C], F32, tag="ptx")
                nc.tensor.matmul(
                    pg[:], rt16[:, t * P : (t + 1) * P], w2[:], start=True, stop=True
                )
                nc.scalar.activation(
                    stage2[:, t, 0:C],
                    pg[:],
                    mybir.ActivationFunctionType.Copy,
                    scale=dinv2_nm[:, t : t + 1],
                )

            edge_phase(hfull, H, b1, H, stage, evac1, gsh_blk)

            # ---- Phase F: layer-2 edges + final transpose/scale ----
            out_stage = stg.tile([P, T, C], F32, tag="ostage")

            def evac2(t, pa):
                sb = io.tile([C, P], F32, tag="ev2")
                nc.vector.tensor_copy(sb[:], pa)
                ptr = pst2.tile([P, C], F32, tag="ptx")
                nc.tensor.transpose(
                    out=ptr[:], in_=sb[:], identity=ident[0:C, 0:C]
                )
                nc.scalar.activation(
                    out_stage[:, t, :],
                    ptr[:],
                    mybir.ActivationFunctionType.Copy,
                    scale=dinv_nm[:, t : t + 1],
                )

            edge_phase(gfull, C, b2, C, stage2, evac2)
            nc.sync.dma_start(out_ext.rearrange("(t p) c -> p t c", p=P)[:], out_stage[:])

    nc.compile()
    return nc


@functools.lru_cache(maxsize=8)
def _build_cached(cfg_key):
    Fdim, H, C, NS, ecnt_t = cfg_key
    return _build((Fdim, H, C, NS, [list(g) for g in ecnt_t]))


# ----------------------------------------------------------------------------
# Host-side sharding / metadata prep
# ----------------------------------------------------------------------------


def _prep(x, edge_index, W1, b1, W2, b2):
    N, Fdim = x.shape
    H = W1.shape[1]
    C = W2.shape[1]
    NS = _round_up(-(-N // NCORE), P)
    T = NS // P
    NPAD = NCORE * NS
    NG = -(-NPAD // GSZ)

    src = np.asarray(edge_index[0], dtype=np.int64)
    dst = np.asarray(edge_index[1], dtype=np.int64)

    deg = np.bincount(dst, minlength=N).astype(np.float32) + 1.0  # + self loop
    deg_pad = np.ones(NPAD, dtype=np.float32)
    deg_pad[:N] = deg

    core = dst // NS
    t_of = (dst % NS) >> 7
    # table rows are laid out [quarter][core][NS/4] so quarter AllGathers
    # produce contiguous output ranges
    QS, QF = NS // 4, NPAD // 4
    spos = src % NS
    scr = src // NS
    sq = spos // QS
    new_row = sq * QF + scr * QS + (spos % QS)
    g_of = new_row >> GSHIFT
    d_of = dst & (P - 1)

    seg_id = (core * NG + g_of) * T + t_of
    cnt = np.bincount(seg_id, minlength=NCORE * NG * T).reshape(NCORE, NG, T)
    ecnt = _round_up(cnt.max(axis=0), 64)  # [NG, T] shared; 64: legal PE base partitions are 0,32,64

    blocks = [list(range(b, min(b + TBSZ, T))) for b in range(0, T, TBSZ)]
    ecnt_l = [[int(v) for v in row] for row in ecnt]
    calls, E_IDX, NCH = _call_layout(ecnt_l, blocks, NG)

    # global idx-stream position of each (g,t) segment
    seg_base = np.zeros((NG, T), dtype=np.int64)
    for bi, blk in enumerate(blocks):
        for g in range(NG):
            info = calls[(bi, g)]
            s = 0
            for t in blk:
                seg_base[g, t] = info["off16"] * 16 + s
                s += ecnt[g, t]

    # rank of each edge inside its (core,g,t) segment
    order = np.argsort(seg_id, kind="stable")
    seg_sorted = seg_id[order]
    starts = np.searchsorted(seg_sorted, np.arange(NCORE * NG * T))
    rank = np.arange(len(order)) - starts[seg_sorted]
    g_sorted = (seg_sorted // T) % NG
    t_sorted = seg_sorted % T
    pos_sorted = seg_base[g_sorted, t_sorted] + rank
    core_sorted = seg_sorted // (NG * T)

    gidx_all = np.zeros((NCORE, E_IDX), dtype=np.int16)
    dloc_all = np.full((NCORE, NCH * P), -1.0, dtype=np.float16)
    # map idx-stream position -> chunk-grid position (call-aligned)
    chunk_pos = np.zeros(max(E_IDX, 1), dtype=np.int64)
    for bi in range(len(blocks)):
        for g in range(NG):
            info = calls[(bi, g)]
            B = info["B"]
            if B == 0:
                continue
            a = info["off16"] * 16
            chunk_pos[a : a + B] = info["offch"] * P + np.arange(B)

    gidx_all[core_sorted, pos_sorted] = (
        new_row[order] - (g_of[order] << GSHIFT)
    ).astype(np.int16)
    dloc_all[core_sorted, chunk_pos[pos_sorted]] = d_of[order].astype(np.float16)

    x_pad = np.zeros((NPAD, Fdim), dtype=np.float32)
    x_pad[:N] = np.asarray(x, dtype=np.float32)

    in_maps = []
    for c in range(NCORE):
        xT = np.ascontiguousarray(x_pad[c * NS : (c + 1) * NS].T)
        dshard = deg_pad[c * NS : (c + 1) * NS]
        deg_nm = np.ascontiguousarray(dshard.reshape(T, P).T)
        deg_row = dshard.reshape(1, NS)
        flat = gidx_all[c]
        gidx_w = np.tile(
            np.ascontiguousarray(flat.reshape(E_IDX // 16, 16).T), (NCORE, 1)
        )
        dloc_w = np.ascontiguousarray(dloc_all[c].reshape(NCH, P).T)
        in_maps.append(
            {
                "xT": xT,
                "deg_nm": deg_nm,
                "deg_row": deg_row,
                "W1": np.asarray(W1, dtype=np.float32).reshape(Fdim, H),
                "W2": np.asarray(W2, dtype=np.float32).reshape(H, C),
                "b1": np.asarray(b1, dtype=np.float32).reshape(1, H),
                "b2": np.asarray(b2, dtype=np.float32).reshape(1, C),
                "gidx": gidx_w,
                "dloc": dloc_w,
            }
        )

    cfg_key = (Fdim, H, C, NS, tuple(tuple(int(v) for v in row) for row in ecnt))
    return cfg_key, in_maps, N, NS, C


def _run(x, edge_index, W1, b1, W2, b2, trace=False):
    cfg_key, in_maps, N, NS, C = _prep(x, edge_index, W1, b1, W2, b2)
    nc = _build_cached(cfg_key)
    res = run_bass_kernel_spmd(nc, in_maps, list(range(NCORE)), trace=trace)
    shards = [res.results[c]["out_nm"] for c in range(NCORE)]
    out = np.concatenate(shards, axis=0)[:N]
    return np.ascontiguousarray(out, dtype=np.float32), res


def kernel(x, edge_index, W1, b1, W2, b2):
    out, _ = _run(x, edge_index, W1, b1, W2, b2)
    return out


# revision 25
# speedup vs baseline: 1.2082x; 1.0317x over previous
"""GCN (2-layer, PyG GCNConv-style) on 8 Trainium2 NeuronCores.

Strategy (sharding_hint): nodes sharded across the 8 cores (data parallel on
the node dim); edges partitioned by destination core so the scatter-add stays
local; per layer the dinv-scaled transformed features are AllGathered so each
core can gather arbitrary source rows; weights replicated.

Math (per layer, A' = A + I, dinv = deg^-1/2):
    out = dinv . (A'^T (dinv . (x @ W))) + b
We fold norms so no per-edge scaling is needed:
  - table  = dinv . (x @ W)                    (per-node scale, ACT)
  - agg    = A^T table + table + b * sqrtdeg   (PE one-hot matmuls; the
             self-loop term "+ table" is a dense per-tile transpose matmul,
             so self loops never enter the gathered edge streams)
  - layer1 h2 = dinv . relu(agg)               (dinv moved past relu, dinv>0)
    the dinv is then folded into layer2's table scale (dinv^2).
  - layer2 out = dinv . agg2                   (final per-node scale)

Edge aggregation: edges are grouped on host by (dst-tile t of 128 nodes,
src-group g of 32768 nodes); per (g,t) the segment is padded to a multiple of
16 only (pad idx=0, pad dst label -1).  Segments are concatenated per
(g, tile-block) into one gather call; matmul chunks of 128 gathered rows may
straddle segment boundaries, which is handled with partition-sliced one-hot
matmuls (the straddle map is static because segment sizes are shared across
cores; per-core count variation hides in the dst labels, where -1 kills the
one-hot).  Scatter-add accumulates directly into one PSUM tile per dst tile.
"""

import functools
import numpy as np

import concourse.bacc as bacc
import concourse.mybir as mybir
import concourse.tile as tile
from concourse.bass_utils import run_bass_kernel_spmd
from concourse.masks import make_identity

NCORE = 8
P = 128
GSHIFT = 15  # src-group size 32768 (int16 index range)
GSZ = 1 << GSHIFT
TBSZ = 4  # dst tiles per gather block

F16 = mybir.dt.float16
F32 = mybir.dt.float32
I16 = mybir.dt.int16


def _round_up(a, b):
    return (a + b - 1) // b * b


def _call_layout(ecnt, blocks, NG):
    """Static stream layout.  Returns per-call info:
    calls[(bi, g)] = dict(B, off16, offch, cov) where cov[k] = list of
    (t, lo, hi) partition-slices of chunk k."""
    calls = {}
    off16 = 0  # global gidx offset, units of 16 idxs
    offch = 0  # global chunk offset (dloc columns)
    for bi, blk in enumerate(blocks):
        for g in range(NG):
            segs = []
            s = 0
            for t in blk:
                e = ecnt[g][t]
                segs.append((t, s, e))
                s += e
            B = s
            K = -(-B // P) if B else 0
            cov = []
            for k in range(K):
                lo_k, hi_k = k * P, (k + 1) * P
                entries = []
                for t, st, e in segs:
                    if e == 0:
                        continue
                    a, b = max(st, lo_k), min(st + e, hi_k)
                    if a < b:
                        entries.append([t, a - lo_k, b - lo_k])
                cov.append(entries)
            calls[(bi, g)] = dict(B=B, off16=off16, offch=offch, cov=cov)
            off16 += B // 16
            offch += K
    return calls, off16 * 16, offch


# ----------------------------------------------------------------------------
# Bass program (identical for all 8 cores; per-core data differs via inputs)
# ----------------------------------------------------------------------------


def _build(cfg):
    """cfg = (F, H, C, NS, ecnt) with ecnt[g][t] = padded16 edge count."""
    Fdim, H, C, NS, ecnt = cfg
    ecnt = [list(gr) for gr in ecnt]
    T = NS // P
    NPAD = NCORE * NS
    NG = len(ecnt)
    blocks = [list(range(b, min(b + TBSZ, T))) for b in range(0, T, TBSZ)]
    calls, E_IDX, NCH = _call_layout(ecnt, blocks, NG)
    SCMAX = max(len(c["cov"]) for c in calls.values())

    nc = bacc.Bacc(None, target_bir_lowering=False)

    # ---- I/O ----
    xT_in = nc.dram_tensor("xT", [P, NS], F16, kind="ExternalInput")
    degnm_in = nc.dram_tensor("deg_nm", [P, T], F32, kind="ExternalInput")
    degrow_in = nc.dram_tensor("deg_row", [1, NS], F32, kind="ExternalInput")
    w1_in = nc.dram_tensor("W1", [Fdim, H], F32, kind="ExternalInput")
    w2_in = nc.dram_tensor("W2", [H, C], F32, kind="ExternalInput")
    b1_in = nc.dram_tensor("b1", [1, H], F32, kind="ExternalInput")
    b2_in = nc.dram_tensor("b2", [1, C], F32, kind="ExternalInput")
    gidx_in = nc.dram_tensor("gidx", [P, E_IDX // 16], I16, kind="ExternalInput")
    dloc_in = nc.dram_tensor("dloc", [P, NCH], F16, kind="ExternalInput")
    out_ext = nc.dram_tensor("out_nm", [NS, C], F32, kind="ExternalOutput")

    hsh = nc.dram_tensor("hsh", [NS, P], F16)
    gsh = nc.dram_tensor("gsh", [NS, P], F16)
    hfull = nc.dram_tensor("hfull", [NPAD, P], F16, addr_space="Shared")
    gfull = nc.dram_tensor("gfull", [NPAD, P], F16, addr_space="Shared")
    rgroups = [list(range(NCORE))]

    with tile.TileContext(nc) as tc:
        with (
            tc.tile_pool(name="con", bufs=1) as con,
            tc.tile_pool(name="meta", bufs=1) as meta,
            tc.tile_pool(name="stg", bufs=1) as stg,
            tc.tile_pool(name="io", bufs=3) as io,
            tc.tile_pool(name="eb", bufs=4) as eb,
            tc.tile_pool(name="ps", bufs=1, space="PSUM") as ps,
            tc.tile_pool(name="pst", bufs=2, space="PSUM") as pst,
            tc.tile_pool(name="pst2", bufs=2, space="PSUM") as pst2,
        ):
            # ---- Phase A: constants / metadata ----
            dloc = meta.tile([P, NCH], F16)
            nc.sync.dma_start(dloc[:], dloc_in[:])

            w1f = con.tile([Fdim, H], F32)
            nc.sync.dma_start(w1f[:], w1_in[:])
            w1 = con.tile([Fdim, H], F16)
            nc.vector.tensor_copy(w1[:], w1f[:])
            w2f = con.tile([H, C], F32)
            nc.sync.dma_start(w2f[:], w2_in[:])
            w2 = con.tile([H, C], F16)
            nc.vector.tensor_copy(w2[:], w2f[:])
            b1f = con.tile([1, H], F32)
            nc.sync.dma_start(b1f[:], b1_in[:])
            b1 = con.tile([1, H], F16)
            nc.vector.tensor_copy(b1[:], b1f[:])
            b2f = con.tile([1, C], F32)
            nc.sync.dma_start(b2f[:], b2_in[:])
            b2 = con.tile([1, C], F16)
            nc.vector.tensor_copy(b2[:], b2f[:])

            degnm = con.tile([P, T], F32)
            nc.sync.dma_start(degnm[:], degnm_in[:])
            sq_nm = con.tile([P, T], F32)
            nc.scalar.activation(sq_nm[:], degnm[:], mybir.ActivationFunctionType.Sqrt)
            dinv_nm = con.tile([P, T], F32)
            nc.vector.reciprocal(dinv_nm[:], sq_nm[:])
            dinv2_nm = con.tile([P, T], F32)
            nc.vector.tensor_mul(dinv2_nm[:], dinv_nm[:], dinv_nm[:])

            degrow = con.tile([1, NS], F32)
            nc.sync.dma_start(degrow[:], degrow_in[:])
            sqrow = con.tile([1, NS], F16)
            nc.scalar.activation(sqrow[:], degrow[:], mybir.ActivationFunctionType.Sqrt)

            iota_i = con.tile([P, P], I16)
            nc.gpsimd.iota(iota_i[:], pattern=[[1, P]], base=0, channel_multiplier=0)
            iota16 = con.tile([P, P], F16)
            nc.vector.tensor_copy(iota16[:], iota_i[:])

            ident = con.tile([P, P], F32)
            make_identity(nc, ident[:])
            ident16 = con.tile([P, P], F16)
            nc.vector.tensor_copy(ident16[:], ident[:])

            stage = stg.tile([P, T, H], F16, tag="stage")
            stage2 = stg.tile([P, T, C], F16, tag="stage2")

            # ---- Phase B: layer-1 transform; quarter AllGathers fire as
            # soon as the tiles covering each table quarter are staged ----
            QS, QF = NS // 4, NPAD // 4

            def ag_quarter(sh, full, q):
                nc.gpsimd.collective_compute(
                    "AllGather",
                    mybir.AluOpType.bypass,
                    ins=[sh[q * QS : (q + 1) * QS, :]],
                    outs=[full[q * QF : (q + 1) * QF, :]],
                    replica_groups=rgroups,
                )

            hq = 0
            hw0 = 0
            for t in range(T):
                xt = io.tile([P, P], F16, tag="xt")
                nc.sync.dma_start(xt[:], xT_in[:, t * P : (t + 1) * P])
                ph = pst.tile([P, H], F32, tag="pt")
                nc.tensor.matmul(ph[:], xt[:], w1[:], start=True, stop=True)
                nc.scalar.activation(
                    stage[:, t, 0:H],
                    ph[:],
                    mybir.ActivationFunctionType.Copy,
                    scale=dinv_nm[:, t : t + 1],
                )
                while hq < 4 and (t + 1) * P >= (hq + 1) * QS:
                    nc.sync.dma_start(
                        hsh.rearrange("(t p) d -> p t d", p=P)[:, hw0 : t + 1, 0:H],
                        stage[:, hw0 : t + 1, :],
                    )
                    hw0 = t + 1
                    ag_quarter(hsh, hfull, hq)
                    hq += 1

            rt16 = stg.tile([H, T * P], F16)

            def edge_phase(table, width, bvec, accw, sstage, evac, block_done=None):
                """psum_t = b*sqrtdeg + table_t^T + A^T table, per dst tile.

                evac(t, psum_tile) consumes the finished accumulation."""
                for bi, blk in enumerate(blocks):
                    psums = {}
                    mm_total = {}
                    mm_done = {}
                    for t in blk:
                        mm_total[t] = 2  # bias + self-loop
                    for g in range(NG):
                        for ent in calls[(bi, g)]["cov"]:
                            for t, lo, hi in ent:
                                mm_total[t] += 1

                    def acc_mm(t, lhsT, rhs):
                        pa = psums[t]
                        k = mm_done[t]
                        nc.tensor.matmul(
                            pa, lhsT, rhs,
                            start=(k == 0), stop=(k == mm_total[t] - 1),
                        )
                        mm_done[t] = k + 1

                    for ti, t in enumerate(blk):
                        pa_t = ps.tile(
                            [accw, P], F32, name=f"acc{ti}", tag=f"acc{ti}"
                        )
                        psums[t] = pa_t[:]
                        mm_done[t] = 0
                        # bias * sqrtdeg row
                        acc_mm(t, bvec[:], sqrow[0:1, t * P : (t + 1) * P])
                        # dense self-loop: psum += table_t^T
                        acc_mm(t, sstage[:, t, 0:width], ident16[:, :])

                    for g in range(NG):
                        info = calls[(bi, g)]
                        B = info["B"]
                        if B == 0:
                            continue
                        sc = len(info["cov"])
                        gbase = g * GSZ
                        gsz = min(GSZ, NPAD - gbase)
                        gi = eb.tile([P, SCMAX * 8], I16, tag="gi")
                        nc.sync.dma_start(
                            gi[:, 0 : B // 16],
                            gidx_in[:, info["off16"] : info["off16"] + B // 16],
                        )
                        msgs = eb.tile([P, SCMAX, P], F16, tag="msgs")
                        nc.gpsimd.dma_gather(
                            msgs[:, 0:sc, :],
                            table[gbase : gbase + gsz, :],
                            gi[:, 0 : B // 16],
                            B,
                            B,
                            P,
                            single_packet=False,
                        )
                        ind = eb.tile([P, SCMAX, P], F16, tag="ind")
                        oc = info["offch"]
                        nc.vector.tensor_tensor(
                            out=ind[:, 0:sc, :],
                            in0=iota16[:, :]
                            .rearrange("p (s d) -> p s d", s=1)
                            .to_broadcast([P, sc, P]),
                            in1=dloc[:, oc : oc + sc]
                            .rearrange("p (s o) -> p s o", o=1)
                            .to_broadcast([P, sc, P]),
                            op=mybir.AluOpType.is_equal,
                        )
                        for k, entries in enumerate(info["cov"]):
                            for t, lo, hi in entries:
                                acc_mm(
                                    t,
                                    msgs[lo:hi, k, 0:width],
                                    ind[lo:hi, k, :],
                                )
                    for ti, t in enumerate(blk):
                        assert mm_done[t] == mm_total[t], (bi, t)
                        evac(t, psums[t])
                    if block_done is not None:
                        block_done(blk)
                    del psums

            # ---- Phase D: layer-1 edges + relu + layer-2 transform ----
            gq_state = [0]

            def gsh_blk(blk):
                t0, t1 = blk[0], blk[-1] + 1
                nc.sync.dma_start(
                    gsh.rearrange("(t p) d -> p t d", p=P)[:, t0:t1, 0:C],
                    stage2[:, t0:t1, :],
                )
                while gq_state[0] < 4 and t1 * P >= (gq_state[0] + 1) * QS:
                    ag_quarter(gsh, gfull, gq_state[0])
                    gq_state[0] += 1

            def evac1(t, pa):
                nc.scalar.activation(
                    rt16[:, t * P : (t + 1) * P],
                    pa,
                    mybir.ActivationFunctionType.Relu,
                )
                pg = pst2.tile([P, C], F32, tag="ptx")
                nc.tensor.matmul(
                    pg[:], rt16[:, t * P : (t + 1) * P], w2[:], start=True, stop=True
                )
                nc.scalar.activation(
                    stage2[:, t, 0:C],
                    pg[:],
                    mybir.ActivationFunctionType.Copy,
                    scale=dinv2_nm[:, t : t + 1],
                )

            edge_phase(hfull, H, b1, H, stage, evac1, gsh_blk)

            # ---- Phase F: layer-2 edges + final transpose/scale ----
            out_stage = stg.tile([P, T, C], F32, tag="ostage")

            def evac2(t, pa):
                sb = io.tile([C, P], F32, tag="ev2")
                nc.vector.tensor_copy(sb[:], pa)
                ptr = pst2.tile([P, C], F32, tag="ptx")
                nc.tensor.transpose(
                    out=ptr[:], in_=sb[:], identity=ident[0:C, 0:C]
                )
                nc.scalar.activation(
                    out_stage[:, t, :],
                    ptr[:],
                    mybir.ActivationFunctionType.Copy,
                    scale=dinv_nm[:, t : t + 1],
                )

            edge_phase(gfull, C, b2, C, stage2, evac2)
            nc.sync.dma_start(out_ext.rearrange("(t p) c -> p t c", p=P)[:], out_stage[:])

    nc.compile()
    return nc


@functools.lru_cache(maxsize=8)
def _build_cached(cfg_key):
    Fdim, H, C, NS, ecnt_t = cfg_key
    return _build((Fdim, H, C, NS, [list(g) for g in ecnt_t]))


# ----------------------------------------------------------------------------
# Host-side sharding / metadata prep
# ----------------------------------------------------------------------------


def _prep(x, edge_index, W1, b1, W2, b2):
    N, Fdim = x.shape
    H = W1.shape[1]
    C = W2.shape[1]
    NS = _round_up(-(-N // NCORE), P)
    T = NS // P
    NPAD = NCORE * NS
    NG = -(-NPAD // GSZ)

    src = np.asarray(edge_index[0], dtype=np.int64)
    dst = np.asarray(edge_index[1], dtype=np.int64)

    deg = np.bincount(dst, minlength=N).astype(np.float32) + 1.0  # + self loop
    deg_pad = np.ones(NPAD, dtype=np.float32)
    deg_pad[:N] = deg

    core = dst // NS
    t_of = (dst % NS) >> 7
    # table rows are laid out [quarter][core][NS/4] so quarter AllGathers
    # produce contiguous output ranges
    QS, QF = NS // 4, NPAD // 4
    spos = src % NS
    scr = src // NS
    sq = spos // QS
    new_row = sq * QF + scr * QS + (spos % QS)
    g_of = new_row >> GSHIFT
    d_of = dst & (P - 1)

    seg_id = (core * NG + g_of) * T + t_of
    cnt = np.bincount(seg_id, minlength=NCORE * NG * T).reshape(NCORE, NG, T)
    ecnt = _round_up(cnt.max(axis=0), 64)  # [NG, T] shared; 64: legal PE base partitions are 0,32,64

    blocks = [list(range(b, min(b + TBSZ, T))) for b in range(0, T, TBSZ)]
    ecnt_l = [[int(v) for v in row] for row in ecnt]
    calls, E_IDX, NCH = _call_layout(ecnt_l, blocks, NG)

    # global idx-stream position of each (g,t) segment
    seg_base = np.zeros((NG, T), dtype=np.int64)
    for bi, blk in enumerate(blocks):
        for g in range(NG):
            info = calls[(bi, g)]
            s = 0
            for t in blk:
                seg_base[g, t] = info["off16"] * 16 + s
                s += ecnt[g, t]

    # rank of each edge inside its (core,g,t) segment
    order = np.argsort(seg_id, kind="stable")
    seg_sorted = seg_id[order]
    starts = np.searchsorted(seg_sorted, np.arange(NCORE * NG * T))
    rank = np.arange(len(order)) - starts[seg_sorted]
    g_sorted = (seg_sorted // T) % NG
    t_sorted = seg_sorted % T
    pos_sorted = seg_base[g_sorted, t_sorted] + rank
    core_sorted = seg_sorted // (NG * T)

    gidx_all = np.zeros((NCORE, E_IDX), dtype=np.int16)
    dloc_all = np.full((NCORE, NCH * P), -1.0, dtype=np.float16)
    # map idx-stream position -> chunk-grid position (call-aligned)
    chunk_pos = np.zeros(max(E_IDX, 1), dtype=np.int64)
    for bi in range(len(blocks)):
        for g in range(NG):
            info = calls[(bi, g)]
            B = info["B"]
            if B == 0:
                continue
            a = info["off16"] * 16
            chunk_pos[a : a + B] = info["offch"] * P + np.arange(B)

    gidx_all[core_sorted, pos_sorted] = (
        new_row[order] - (g_of[order] << GSHIFT)
    ).astype(np.int16)
    dloc_all[core_sorted, chunk_pos[pos_sorted]] = d_of[order].astype(np.float16)

    x_pad = np.zeros((NPAD, Fdim), dtype=np.float32)
    x_pad[:N] = np.asarray(x, dtype=np.float32)

    in_maps = []
    for c in range(NCORE):
        xT = np.ascontiguousarray(x_pad[c * NS : (c + 1) * NS].T.astype(np.float16))
        dshard = deg_pad[c * NS : (c + 1) * NS]
        deg_nm = np.ascontiguousarray(dshard.reshape(T, P).T)
        deg_row = dshard.reshape(1, NS)
        flat = gidx_all[c]
        gidx_w = np.tile(
            np.ascontiguousarray(flat.reshape(E_IDX // 16, 16).T), (NCORE, 1)
        )
        dloc_w = np.ascontiguousarray(dloc_all[c].reshape(NCH, P).T)
        in_maps.append(
            {
                "xT": xT,
                "deg_nm": deg_nm,
                "deg_row": deg_row,
                "W1": np.asarray(W1, dtype=np.float32).reshape(Fdim, H),
                "W2": np.asarray(W2, dtype=np.float32).reshape(H, C),
                "b1": np.asarray(b1, dtype=np.float32).reshape(1, H),
                "b2": np.asarray(b2, dtype=np.float32).reshape(1, C),
                "gidx": gidx_w,
                "dloc": dloc_w,
            }
        )

    cfg_key = (Fdim, H, C, NS, tuple(tuple(int(v) for v in row) for row in ecnt))
    return cfg_key, in_maps, N, NS, C


def _run(x, edge_index, W1, b1, W2, b2, trace=False):
    cfg_key, in_maps, N, NS, C = _prep(x, edge_index, W1, b1, W2, b2)
    nc = _build_cached(cfg_key)
    res = run_bass_kernel_spmd(nc, in_maps, list(range(NCORE)), trace=trace)
    shards = [res.results[c]["out_nm"] for c in range(NCORE)]
    out = np.concatenate(shards, axis=0)[:N]
    return np.ascontiguousarray(out, dtype=np.float32), res


def kernel(x, edge_index, W1, b1, W2, b2):
    out, _ = _run(x, edge_index, W1, b1, W2, b2)
    return out
